# revision 24
# baseline (speedup 1.0000x reference)
"""AttentionHead kernel for 8 TRN2 NeuronCores.

Problem: q = x@Wq+bq; k = y@Wk+bk; v = y@Wv+bv
         att = softmax(q k^T / sqrt(128));  att = triu(att, k=1)  (AFTER softmax)
         out = att @ v
Shapes: x [4, 2048, 1024], y [4, 2048, 1024], W* [1024, 128], out [4, 2048, 128].

Sharding: 8 cores = (batch b in 0..3) x (query-half h in 0..1). Core (b, h)
computes queries [h*1024, (h+1)*1024) of batch b against all 2048 keys.
No cross-core communication.

SPMD uniformity trick: the post-softmax causal mask (keep key j > query i)
depends on the query offset h*1024, which differs per core, but all cores
must run the SAME graph. We rotate the key axis per core on host
(j_local = (j_global - h*1024) mod 2048). Then for every core:
  - keys j_local in [0, 1024): keep iff j_local > i_local  (same triangular
    band for every core -> one compile-time mask input shared by all cores)
  - keys j_local in [1024, 2048): keep-all for h=0, drop-all for h=1 ->
    handled by scaling those V tiles by a per-core scalar g in {1.0, 0.0}.
The softmax normalizer sums exp over ALL keys (mask comes after softmax),
and is invariant to the key rotation.

On-chip layout: host pre-transposes x/y to [feature, seq] bf16 so the
projections produce qT [d, i] / kT [d, j] / vT [d, j] directly in the
layouts the PE array needs; vT is PE-transposed to v [j, d] tiles.
Scores are computed transposed, ST [j, i]; Z[i] = sum_j exp(ST) via a
ones-stationary matmul (broadcast over partitions); O^T [d, i] =
sum_j v[j, d]^T . maskedexp[j, i]; final scale by 1/Z on DVE.
Host transposes O^T back.

The k/v projections, V transposes and the attention t-loop are fused
along key chunks of 512 so ACT's exp stream overlaps the projection
matmuls instead of running after them.
"""

import numpy as np
import ml_dtypes

B = 4
LQ = 2048
LK = 2048
XS = 1024
PD = 128
LQS = LQ // 2  # queries per core: 1024

NE = XS // 128  # 8 contraction tiles for projections
NT = LK // 128  # 16 key tiles
CH = 512  # chunk (PSUM bank = 512 f32)
NCH = LQS // CH  # 2 query chunks
NKC = LK // CH  # 4 key chunks
SM_SCALE = 1.0 / float(np.sqrt(PD))

_BF16 = ml_dtypes.bfloat16

_graph_cache = {}


def _build_graph(apply_mask: bool):
    import concourse.mybir as mybir
    from concourse import bacc
    from concourse.masks import make_identity
    from concourse.tile import TileContext

    BF = mybir.dt.bfloat16
    F32 = mybir.dt.float32
    Exp = mybir.ActivationFunctionType.Exp
    Identity = mybir.ActivationFunctionType.Identity

    nc = bacc.Bacc()

    xT = nc.declare_dram_parameter("xT", [XS, LQS], BF, isOutput=False)
    yT = nc.declare_dram_parameter("yT", [XS, LK], BF, isOutput=False)
    Wq = nc.declare_dram_parameter("Wq", [XS, PD], BF, isOutput=False)
    Wk = nc.declare_dram_parameter("Wk", [XS, PD], BF, isOutput=False)
    Wv = nc.declare_dram_parameter("Wv", [XS, PD], BF, isOutput=False)
    # Packed small constants: cols 0..2 = bq, bk, bv; cols 3..18 = gv
    # (per-v-tile scale g: cols 0..7 = 1; cols 8..15 = 1 or 0 per core,
    # broadcast over the 128 partitions).
    consts = nc.declare_dram_parameter("consts", [128, 3 + NT], F32, isOutput=False)
    if apply_mask:
        # tri[jj, c] = 1.0 if jj > c - 384 else 0.0, c in [0, 896).
        # Band mask for key-tile t vs query chunk c: delta = 128*t - 512*c,
        # slice cols [384-delta, 384-delta+512).
        tri = nc.declare_dram_parameter("tri", [128, 896], BF, isOutput=False)
    out_ext = nc.declare_dram_parameter("out", [PD, LQS], F32, isOutput=True)

    with TileContext(nc) as tc:
        with (
            tc.tile_pool(name="const", bufs=1) as const_pool,
            tc.tile_pool(name="sb", bufs=1) as sb_pool,
            tc.tile_pool(name="exp", bufs=3) as exp_pool,
            tc.tile_pool(name="ps", bufs=2, space="PSUM") as ps_pool,
            tc.tile_pool(name="psacc", bufs=1, space="PSUM") as psacc_pool,
        ):
            # ---- input DMAs: few big instructions (each dma_start costs the
            # issuing sequencer ~0.7-1.4us of serial DIRECT2D time), spread
            # across the two HWDGE-capable sequencers (SP for the bulk x/y
            # stream, ACT for weights + small constants) and ordered so the
            # first-needed tensors land first.
            Wq_sb = sb_pool.tile([128, NE, PD], BF)
            Wk_sb = sb_pool.tile([128, NE, PD], BF)
            Wv_sb = sb_pool.tile([128, NE, PD], BF)
            xT_sb = sb_pool.tile([128, NE, LQS], BF)
            yT_sb = sb_pool.tile([128, NE, LK], BF)
            consts_sb = const_pool.tile([128, 3 + NT], F32)
            if apply_mask:
                tri_sb = const_pool.tile([128, 896], BF)
            xT_r = xT.rearrange("(e p) i -> p e i", p=128)
            yT_r = yT.rearrange("(e p) i -> p e i", p=128)
            nc.scalar.dma_start(out=Wq_sb, in_=Wq.rearrange("(e p) d -> p e d", p=128))
            nc.scalar.dma_start(out=Wk_sb, in_=Wk.rearrange("(e p) d -> p e d", p=128))
            nc.scalar.dma_start(out=Wv_sb, in_=Wv.rearrange("(e p) d -> p e d", p=128))
            nc.scalar.dma_start(out=consts_sb, in_=consts[:, :])
            if apply_mask:
                nc.scalar.dma_start(out=tri_sb, in_=tri[:, :])
            nc.sync.dma_start(out=xT_sb[:, 0:4, 0:CH], in_=xT_r[:, 0:4, 0:CH])
            nc.sync.dma_start(out=xT_sb[:, 4:8, 0:CH], in_=xT_r[:, 4:8, 0:CH])
            nc.sync.dma_start(out=yT_sb[:, 0:4, 0:CH], in_=yT_r[:, 0:4, 0:CH])
            nc.sync.dma_start(out=yT_sb[:, 4:8, 0:CH], in_=yT_r[:, 4:8, 0:CH])
            nc.sync.dma_start(out=xT_sb[:, :, CH:LQS], in_=xT_r[:, :, CH:LQS])
            for c in range(1, NKC):
                cs = slice(c * CH, (c + 1) * CH)
                nc.sync.dma_start(out=yT_sb[:, :, cs], in_=yT_r[:, :, cs])

            bq_sb = consts_sb[:, 0:1]
            bk_sb = consts_sb[:, 1:2]
            bv_sb = consts_sb[:, 2:3]
            gv_sb = consts_sb[:, 3:]

            # ---- constants, identity, ACT table prime ----
            ones_sb = const_pool.tile([128, 128], BF)
            nc.vector.memset(ones_sb, 1.0)
            ident_sb = const_pool.tile([128, 128], BF)
            make_identity(nc, ident_sb)
            # Touch Exp early (after the ACT-queue DMA issues) so the ~1.3us
            # ACT_TABLE_LOAD overlaps the input DMAs.
            scratch1 = const_pool.tile([1, 1], F32)
            nc.scalar.activation(scratch1, ones_sb[0:1, 0:1], Exp)
            # PE warm-up: ~3.5us of dummy matmuls during the DMA lead-in so
            # the HAM clock gate reaches 8/8 before the first real matmul
            # (cold matmuls run at 1.2 instead of 2.4 GHz).
            warm_rhs = const_pool.tile([128, CH], BF)
            nc.vector.memset(warm_rhs, 1.0)
            warm_ps = psacc_pool.tile([128, CH], mybir.dt.float32, tag="z0")
            for _ in range(16):
                nc.tensor.matmul(warm_ps, lhsT=ones_sb, rhs=warm_rhs,
                                 start=True, stop=True)

            # ---- qT projection [d, i] (bias on DVE; bf16 out) ----
            qT_sb = sb_pool.tile([128, LQS], BF)
            for c in range(NCH):
                cs = slice(c * CH, (c + 1) * CH)
                ps = ps_pool.tile(
                    [128, CH], mybir.dt.float32, tag="rot", bufs=4, name="qps"
                )
                for e in range(NE):
                    nc.tensor.matmul(
                        ps,
                        lhsT=Wq_sb[:, e, :],
                        rhs=xT_sb[:, e, cs],
                        start=(e == 0),
                        stop=(e == NE - 1),
                    )
                nc.vector.tensor_scalar_add(qT_sb[:, cs], ps, bq_sb)

            kT_sb = sb_pool.tile([128, LK], BF)
            vT_sb = sb_pool.tile([128, LK], BF)
            v_sb = sb_pool.tile([128, NT, PD], BF)
            z_ps = [
                psacc_pool.tile(
                    [128, CH], mybir.dt.float32, tag=f"z{c}", name=f"z_ps{c}"
                )
                for c in range(NCH)
            ]
            o_ps = [
                psacc_pool.tile(
                    [128, CH], mybir.dt.float32, tag=f"o{c}", name=f"o_ps{c}"
                )
                for c in range(NCH)
            ]

            # ---- fused along key chunks: kT/vT projections, V transposes,
            # then the attention t-loop for the chunk's 4 key tiles.
            # All rotating PSUM tiles share one-bank slots (tag "rot").
            # The kT projection runs one chunk ahead (software pipeline) so
            # its PSUM->SBUF copy latency hides under the previous chunk's
            # score matmuls.
            # The softmax normalizer Z is computed cheaply: the 4 exp tiles of
            # a key chunk are summed on DVE in bf16 (E_kc), and only E_kc goes
            # through the ones-matmul, accumulating across chunks in f32 PSUM.
            def emit_ktproj(kc):
                ks = slice(kc * CH, (kc + 1) * CH)
                ps = ps_pool.tile(
                    [128, CH], mybir.dt.float32, tag="rot", bufs=4, name="kps"
                )
                for e in range(NE):
                    nc.tensor.matmul(
                        ps,
                        lhsT=Wk_sb[:, e, :],
                        rhs=yT_sb[:, e, ks],
                        start=(e == 0),
                        stop=(e == NE - 1),
                    )
                nc.scalar.activation(kT_sb[:, ks], ps, Identity, bias=bk_sb)

            recip_sb = sb_pool.tile([128, LQS], mybir.dt.float32)
            out_sb = sb_pool.tile([128, LQS], mybir.dt.float32)

            def emit_st_exp(t, c, ek, e_tiles):
                cs = slice(c * CH, (c + 1) * CH)
                ts_ = slice(t * 128, (t + 1) * 128)
                st = ps_pool.tile(
                    [128, CH], mybir.dt.float32, tag="rot", bufs=4, name="st"
                )
                # ST [j, i] = kT_t^T qT (full d contraction, one shot)
                nc.tensor.matmul(
                    st, lhsT=kT_sb[:, ts_], rhs=qT_sb[:, cs], start=True, stop=True
                )
                e_sb = exp_pool.tile([128, CH], BF, bufs=10, name="e_sb")
                nc.scalar.activation(e_sb, st, Exp, scale=SM_SCALE)
                e_tiles[t, c] = e_sb
                # chunk-local unmasked sum for the normalizer
                if t % 4 == 0:
                    nc.vector.tensor_copy(ek[c], e_sb)
                else:
                    nc.vector.tensor_add(ek[c], ek[c], e_sb)
                if apply_mask and t < 8 and t // 4 == c:
                    off = 384 - (128 * t - CH * c)
                    nc.vector.tensor_mul(e_sb, e_sb, tri_sb[:, off:off + CH])

            def emit_vproj(kc):
                ks = slice(kc * CH, (kc + 1) * CH)
                # vT chunk (bias on DVE)
                ps = ps_pool.tile(
                    [128, CH], mybir.dt.float32, tag="rot", bufs=4, name="vps"
                )
                for e in range(NE):
                    nc.tensor.matmul(
                        ps,
                        lhsT=Wv_sb[:, e, :],
                        rhs=yT_sb[:, e, ks],
                        start=(e == 0),
                        stop=(e == NE - 1),
                    )
                nc.vector.tensor_scalar_add(vT_sb[:, ks], ps, bv_sb)
                # V transposes for this chunk (g scale on the way out)
                for t in range(4 * kc, 4 * kc + 4):
                    pst = ps_pool.tile([128, PD], BF, tag="rot", bufs=4, name="pst")
                    nc.tensor.transpose(
                        pst, vT_sb[:, t * 128:(t + 1) * 128], ident_sb
                    )
                    nc.vector.tensor_scalar_mul(v_sb[:, t, :], pst, gv_sb[:, t:t + 1])

            def emit_p5(t, c, e_tiles):
                # O^T [d, i] += v_t^T @ maskedexp ; skip all-zero tiles
                if (not apply_mask) or t >= 4 * c:
                    first_t = 4 * c if apply_mask else 0
                    nc.tensor.matmul(
                        o_ps[c],
                        lhsT=v_sb[:, t, :],
                        rhs=e_tiles[t, c],
                        start=(t == first_t),
                        stop=(t == NT - 1),
                    )

            def emit_z(kc, c, ek):
                # Z += ones^T @ E_kc (broadcast row-sum over partitions)
                nc.tensor.matmul(
                    z_ps[c],
                    lhsT=ones_sb,
                    rhs=ek[c],
                    start=(kc == 0),
                    stop=(kc == NKC - 1),
                )

            def emit_finalize(c):
                cs = slice(c * CH, (c + 1) * CH)
                nc.vector.reciprocal_approx_fast(recip_sb[:, cs], z_ps[c])
                nc.vector.tensor_mul(out_sb[:, cs], o_ps[c], recip_sb[:, cs])
                nc.sync.dma_start(out=out_ext[:, cs], in_=out_sb[:, cs])

            emit_ktproj(0)
            for kc in range(NKC - 1):
                ek = [
                    exp_pool.tile([128, CH], BF, tag=f"ek{c}", name=f"ek{c}")
                    for c in range(NCH)
                ]
                e_tiles = {}
                for t in range(4 * kc, 4 * kc + 4):
                    for c in range(NCH):
                        emit_st_exp(t, c, ek, e_tiles)
                # next chunk's kT projection (pipelined ahead)
                emit_ktproj(kc + 1)
                emit_vproj(kc)
                for t in range(4 * kc, 4 * kc + 4):
                    for c in range(NCH):
                        emit_p5(t, c, e_tiles)
                for c in range(NCH):
                    emit_z(kc, c, ek)

            # last key chunk runs query-chunk-major so chunk 0's normalize +
            # store overlap chunk 1's scores/exps
            kc = NKC - 1
            ek = [
                exp_pool.tile([128, CH], BF, tag=f"ek{c}", name=f"ek{c}")
                for c in range(NCH)
            ]
            e_tiles = {}
            for c in range(NCH):
                for t in range(4 * kc, 4 * kc + 4):
                    emit_st_exp(t, c, ek, e_tiles)
                if c == 0:
                    emit_vproj(kc)
                for t in range(4 * kc, 4 * kc + 4):
                    emit_p5(t, c, e_tiles)
                emit_z(kc, c, ek)
                emit_finalize(c)

    nc.finalize()
    return nc


def _get_graph(apply_mask: bool):
    key = bool(apply_mask)
    if key not in _graph_cache:
        _graph_cache[key] = _build_graph(key)
    return _graph_cache[key]


def kernel(**inputs) -> np.ndarray:
    from concourse.bass_utils import run_bass_kernel_spmd

    x = np.asarray(inputs["x"], dtype=np.float32)
    y = np.asarray(inputs["y"], dtype=np.float32)
    Wq = np.asarray(inputs["Wq"], dtype=np.float32)
    Wk = np.asarray(inputs["Wk"], dtype=np.float32)
    Wv = np.asarray(inputs["Wv"], dtype=np.float32)
    bq = np.asarray(inputs["bq"], dtype=np.float32)
    bk = np.asarray(inputs["bk"], dtype=np.float32)
    bv = np.asarray(inputs["bv"], dtype=np.float32)
    mask = bool(np.asarray(inputs["mask"]).item())

    nc = _get_graph(mask)

    Wq_b = Wq.astype(_BF16)
    Wk_b = Wk.astype(_BF16)
    Wv_b = Wv.astype(_BF16)

    if mask:
        cc = np.arange(896, dtype=np.int64)[None, :] - 384
        jj = np.arange(128, dtype=np.int64)[:, None]
        tri = (jj > cc).astype(_BF16)

    in_maps = []
    for core in range(8):
        b, h = core // 2, core % 2
        qoff = h * LQS
        xs = x[b, qoff:qoff + LQS, :]
        ys = np.roll(y[b], -qoff, axis=0) if qoff else y[b]
        g = 1.0 if (h == 0 or not mask) else 0.0
        consts_arr = np.ones((128, 3 + NT), dtype=np.float32)
        consts_arr[:, 0] = bq
        consts_arr[:, 1] = bk
        consts_arr[:, 2] = bv
        consts_arr[:, 3 + NT // 2:] = g
        m = {
            "xT": np.ascontiguousarray(xs.T).astype(_BF16),
            "yT": np.ascontiguousarray(ys.T).astype(_BF16),
            "Wq": Wq_b,
            "Wk": Wk_b,
            "Wv": Wv_b,
            "consts": consts_arr,
        }
        if mask:
            m["tri"] = tri
        in_maps.append(m)

    res = run_bass_kernel_spmd(nc, in_maps, core_ids=list(range(8)))

    out = np.empty((B, LQ, PD), dtype=np.float32)
    for core in range(8):
        b, h = core // 2, core % 2
        qoff = h * LQS
        out[b, qoff:qoff + LQS, :] = res.results[core]["out"].T
    return out
